# revision 1
# baseline (speedup 1.0000x reference)
"""Trainium2 Bass kernel for a dense transformer block (nn_Block_7911329760080).

Reference computation (B=2, T=2048 tokens, C=1024 channels, 16 heads, fp32):
    x = x + Attn(LN1(x));  x = x + MLP(LN2(x))   [full non-causal attention]

Sharding: token-parallel over 8 cores (4 cores per batch, 512 tokens each).
Each core recomputes LN1 stats + K + V for its whole batch (no collectives);
Q / attention / MLP only for its own 512-token slice.

Layout: activations are feature-major ([feature, token], "T" suffix) so every
matmul (out = lhsT.T @ rhs, contraction on partitions) directly produces the
layout the next one consumes — zero on-device transposes.

LN1 is FOLDED into the Q/K/V projections: LN(x) = a_t * x + c_t per token
(with gamma/beta absorbed into the weights on the host), and since the
projections are linear, proj(LN(x)) = a_t * proj(x) + c_t * colsum(W') + bias.
The projections therefore run on RAW x (available immediately), and the tiny
rank-1 fixup happens during PSUM evacuation.  LN statistics come from
ones-vector matmuls; softmax is max-free (scores are small) with the per-query
normalizer produced free by an interleaved ones-column in V during the P@V
matmul.  All matmuls run in float32r (full PE rate at moving dim >= 256).
The attention inner loop is software-pipelined: scores run one step ahead of
P@V, and the next head-pair's K-projection matmuls fill the exp-wait slack.
"""

import numpy as np
import sys
from contextlib import ExitStack

sys.path.insert(0, "/opt/trn_rl_repo/concourse")
sys.path.insert(0, "/opt/trn_rl_repo")

import concourse.bass as bass
import concourse.bacc as bacc
import concourse.mybir as mybir
import concourse.tile as tile

F32 = mybir.dt.float32
F32R = mybir.dt.float32r
ACTF = mybir.ActivationFunctionType
ALU = mybir.AluOpType

N_CORES = 8
B, T, C = 2, 2048, 1024
NH, HD = 16, 64
TQ = T * B // N_CORES          # 512 tokens per core
HID = 4 * C                    # 4096
NCT = C // 128                 # 8 c-tiles
NHT = HID // 128               # 32 hidden-dim tiles
NTT = T // 128                 # 16 token-tiles (full batch)
NCH = T // 512                 # 4 chunks (a chunk = 512 tokens)
LN_EPS = 1e-5

# colpack column layout ([128, n] per-partition bias/scale columns)
CP_BQ8, CP_BK, CP_BO, CP_B2, CP_G1, CP_BL1, CP_G2, CP_BL2, CP_BV = (
    0, 8, 16, 24, 32, 40, 48, 56, 64)
CP_B1 = 72          # 32 cols
CP_EPS, CP_NEG1, CP_QSCL = 104, 105, 106
CP_CWK, CP_CWQ8 = 107, 115
CP_N = 123

_CACHE = {}


def _pack_cols(vec):
    """[n*128] -> [128, n]: column j holds vec[128j:128j+128]."""
    return np.ascontiguousarray(vec.astype(np.float32).reshape(-1, 128).T)


def _grid(w):
    K, F = w.shape
    return np.ascontiguousarray(
        w.reshape(K // 128, 128, F // 128, 128).transpose(0, 2, 1, 3))


def _grid4(w):
    K, F = w.shape
    return np.ascontiguousarray(
        w.reshape(K // 128, 128, F // 512, 512).transpose(0, 2, 1, 3))


def _grid2(w):
    K, F = w.shape
    return np.ascontiguousarray(
        w.reshape(K // 128, 128, F // 256, 256).transpose(0, 2, 1, 3))


def _build_program():
    nc = bacc.Bacc("TRN2", target_bir_lowering=False, debug=False,
                   num_devices=N_CORES)

    def din(name, shape):
        return nc.dram_tensor(name, list(shape), F32, kind="ExternalInput")

    xbT = din("xbT", (C, T))            # full batch, feature-major (raw)
    xqT = din("xqT", (C, TQ))           # own token slice, feature-major (raw)
    wq_p = din("wq_p", (NCT, 4, 128, 256))
    wk_g = din("wk_g", (NCT, NCT, 128, 128))
    wv = din("wv", (C, C))              # gamma-scaled, natural layout
    wo_p = din("wo_p", (NCT, 2, 128, 512))
    w1_p = din("w1_p", (NCT, 8, 128, 512))
    w2_p = din("w2_p", (NHT, 2, 128, 512))
    rowwv = din("rowwv", (C,))          # colsum of gamma-scaled Wv
    colpack = din("colpack", (128, CP_N))
    out_d = nc.dram_tensor("outT", [C, TQ], F32, kind="ExternalOutput")
    acr_d = nc.dram_tensor("acr", [2, T], F32)   # bounce for a/c scatter
    x2_d = nc.dram_tensor("x2d", [C, TQ], F32)   # x + attn(LN1(x)), spilled

    with tile.TileContext(nc) as tc, ExitStack() as top:
        consts = top.enter_context(tc.tile_pool(name="consts", bufs=1))

        cp = consts.tile([128, CP_N], F32)
        nc.sync.dma_start(out=cp, in_=colpack.ap())
        ones_col = consts.tile([128, 1], F32R)
        nc.vector.memset(ones_col.bitcast(F32), 1.0)
        ones_row = consts.tile([1, 128], F32R)
        nc.vector.memset(ones_row.bitcast(F32), 1.0)
        rw_bc = consts.tile([128, C], F32)
        rw_src = rowwv.ap()
        rw_src = bass.AP(tensor=rw_src.tensor, offset=rw_src.offset,
                         ap=[[0, 128]] + list(rw_src.ap))
        nc.sync.dma_start(out=rw_bc, in_=rw_src)

        def col(idx):
            return cp[:, idx:idx + 1]

        def row_const(idx):
            return cp[0:1, idx:idx + 1]

        # ---------------- LayerNorm stats (feature-major) ----------------
        def make_ln_pools(st, name, psum=True):
            pools = {}
            if psum:
                pools["ps_st"] = st.enter_context(
                    tc.tile_pool(name=f"{name}_ps", bufs=2, space="PSUM"))
                pools["ps_bc"] = st.enter_context(
                    tc.tile_pool(name=f"{name}_pb", bufs=1, space="PSUM"))
            pools["work"] = st.enter_context(tc.tile_pool(name=f"{name}_w", bufs=3))
            pools["rows"] = st.enter_context(tc.tile_pool(name=f"{name}_r", bufs=1))
            pools["bc"] = st.enter_context(tc.tile_pool(name=f"{name}_b", bufs=1))
            return pools

        def src_tile(src, ct, ch):
            e = src[ct]
            return e[ch] if isinstance(e, list) else e

        def ln_stats(src, Tn, tag, pools, bc_pool, scale8=False, cols=False):
            """src: [ct] or [ct][ch] SBUF tiles [128,512] F32R (raw x).
            Computes LN(x) = a_t*x + c_t.  Returns dict with per-chunk
            broadcast tiles 'a' (or 'a8' = a/8), 'c', and optionally
            token-major column tiles 'acol'/'ccol' [128, Tn/128]."""
            nch = Tn // 512
            rows, work = pools["rows"], pools["work"]
            res = {"a": [], "c": []}
            arow = crow = None
            if cols:
                arow = rows.tile([1, Tn], F32, tag=f"{tag}arow")
                crow = rows.tile([1, Tn], F32, tag=f"{tag}crow")
            for ch in range(nch):
                sl = slice(ch * 512, ch * 512 + 512)
                ps_s = pools["ps_st"].tile([1, 512], F32, tag="ps_s")
                ps_q = pools["ps_st"].tile([1, 512], F32, tag="ps_q")
                xcs = [src_tile(src, ct, ch) for ct in range(NCT)]
                sqs = []
                for ct in range(NCT):
                    sq = work.tile([128, 512], F32R, tag="sq")
                    nc.vector.tensor_mul(sq, xcs[ct].bitcast(F32),
                                         xcs[ct].bitcast(F32))
                    sqs.append(sq)
                for ct in range(NCT):
                    nc.tensor.matmul(ps_s, ones_col, xcs[ct],
                                     start=(ct == 0), stop=(ct == NCT - 1))
                for ct in range(NCT):
                    nc.tensor.matmul(ps_q, ones_col, sqs[ct],
                                     start=(ct == 0), stop=(ct == NCT - 1))
                mu = rows.tile([1, 512], F32, tag=f"{tag}mu")
                nc.vector.tensor_scalar_mul(mu, ps_s, 1.0 / C)
                msq = rows.tile([1, 512], F32, tag=f"{tag}msq")
                nc.vector.tensor_scalar_mul(msq, ps_q, 1.0 / C)
                mu2 = rows.tile([1, 512], F32, tag=f"{tag}mu2")
                nc.vector.tensor_mul(mu2, mu, mu)
                nc.vector.tensor_sub(msq, msq, mu2)
                std = rows.tile([1, 512], F32, tag=f"{tag}std")
                nc.scalar.activation(std, msq, ACTF.Sqrt, bias=row_const(CP_EPS))
                nc.vector.reciprocal_approx_fast(out=std, in_=std)  # std <- rstd
                nc.vector.tensor_mul(mu, mu, std)                   # mu <- mu*rstd
                if cols:
                    nc.vector.tensor_copy(arow[:, sl], std)
                    nc.vector.tensor_scalar_mul(crow[:, sl], mu, -1.0)
                if scale8:
                    nc.vector.tensor_scalar_mul(std, std, 0.125)
                rstd_r = rows.tile([1, 512], F32R, tag=f"{tag}rr")
                nc.scalar.activation(rstd_r, std, ACTF.Copy)
                nmu_r = rows.tile([1, 512], F32R, tag=f"{tag}nm")
                nc.scalar.activation(nmu_r, mu, ACTF.Copy, scale=row_const(CP_NEG1))
                ps_a = pools["ps_bc"].tile([128, 512], F32, tag="ps_a")
                nc.tensor.matmul(ps_a, ones_row, rstd_r, start=True, stop=True)
                a_bc = bc_pool.tile([128, 512], F32, tag=f"{tag}a{ch}")
                nc.vector.tensor_copy(a_bc, ps_a)
                res["a"].append(a_bc)
                ps_c = pools["ps_bc"].tile([128, 512], F32, tag="ps_c")
                nc.tensor.matmul(ps_c, ones_row, nmu_r, start=True, stop=True)
                c_bc = bc_pool.tile([128, 512], F32, tag=f"{tag}c{ch}")
                nc.vector.tensor_copy(c_bc, ps_c)
                res["c"].append(c_bc)
            if cols:
                # scatter [1,Tn] rows to token-major [128, Tn/128] columns
                # via a DRAM bounce (free-axis -> partition-axis move)
                nc.sync.dma_start(out=acr_d.ap()[0, 0:Tn], in_=arow)
                nc.sync.dma_start(out=acr_d.ap()[1, 0:Tn], in_=crow)
                acol = bc_pool.tile([128, Tn // 128], F32, tag=f"{tag}acol")
                nc.sync.dma_start(
                    out=acol,
                    in_=acr_d.ap()[0, 0:Tn].rearrange("(tt p) -> p tt", p=128))
                ccol = bc_pool.tile([128, Tn // 128], F32, tag=f"{tag}ccol")
                nc.sync.dma_start(
                    out=ccol,
                    in_=acr_d.ap()[1, 0:Tn].rearrange("(tt p) -> p tt", p=128))
                res["acol"], res["ccol"] = acol, ccol
            return res

        def ln_apply(src, Tn, tag, g_idx, b_idx, out_pool, out_tag, pools, abc):
            a_bcs, c_bcs = abc["a"], abc["c"]
            nch = Tn // 512
            work = pools["work"]
            outs = [[None] * nch for _ in range(NCT)]
            for ch in range(nch):
                for ct in range(NCT):
                    o = out_pool.tile([128, 512], F32R, tag=f"{out_tag}{ct}_{ch}")
                    xc = src_tile(src, ct, ch)
                    t1 = work.tile([128, 512], F32, tag="t1")
                    nc.vector.tensor_mul(t1, xc.bitcast(F32), a_bcs[ch])
                    t2 = work.tile([128, 512], F32, tag="t2")
                    nc.vector.tensor_add(t2, t1, c_bcs[ch])
                    nc.scalar.activation(o, t2, ACTF.Identity,
                                         scale=col(g_idx + ct), bias=col(b_idx + ct))
                    outs[ct][ch] = o
            return outs

        # persistent pool: raw x tiles, qT, LN1 fixup tiles (to attention end)
        sp1 = top.enter_context(ExitStack())
        p1 = sp1.enter_context(tc.tile_pool(name="p1", bufs=1))

        # raw activations, feature-major, chunked [128,512]
        xT = []
        for ct in range(NCT):
            per_ct = []
            for ch in range(NCH):
                xt_c = p1.tile([128, 512], F32R, tag=f"xT{ct}_{ch}")
                nc.sync.dma_start(
                    out=xt_c,
                    in_=xbT.ap()[ct * 128:(ct + 1) * 128,
                                 ch * 512:ch * 512 + 512].bitcast(F32R))
                per_ct.append(xt_c)
            xT.append(per_ct)
        # -------- Phases 1-2: LN stats and folded Q projection --------
        qT = []
        with ExitStack() as stln:
            xqp = stln.enter_context(tc.tile_pool(name="xqp", bufs=1))
            xq_raw = []
            for ct in range(NCT):
                xt_q = xqp.tile([128, 512], F32R, tag=f"xq{ct}")
                nc.sync.dma_start(
                    out=xt_q, in_=xqT.ap()[ct * 128:(ct + 1) * 128, :].bitcast(F32R))
                xq_raw.append(xt_q)
            pools = make_ln_pools(stln, "ln0")
            lnB = ln_stats(xq_raw, TQ, "B", pools, pools["bc"], scale8=True)
            lnA = ln_stats(xT, T, "A", pools, p1, cols=True)
            # Q projection on raw xq; LN fold + bias + 1/8 scale in the evac
            wpool = stln.enter_context(tc.tile_pool(name="wq", bufs=3))
            qps = stln.enter_context(tc.tile_pool(name="qps", bufs=1, space="PSUM"))
            qwork = stln.enter_context(tc.tile_pool(name="qw", bufs=2))
            for grp in range(4):
                pss = []
                for f2 in range(2):
                    ps = qps.tile([128, TQ], F32, tag=f"ps{f2}")
                    pss.append(ps)
                for ct in range(NCT):
                    wt = wpool.tile([128, 256], F32R, tag="w")
                    nc.sync.dma_start(out=wt, in_=wq_p.ap()[ct, grp].bitcast(F32R))
                    for f2 in range(2):
                        nc.tensor.matmul(pss[f2], wt[:, f2 * 128:f2 * 128 + 128],
                                         xq_raw[ct], start=(ct == 0),
                                         stop=(ct == NCT - 1))
                for f2 in range(2):
                    ft = grp * 2 + f2
                    o1 = qwork.tile([128, TQ], F32, tag="o1")
                    nc.vector.tensor_scalar(out=o1, in0=lnB["c"][0],
                                            scalar1=col(CP_CWQ8 + ft),
                                            scalar2=col(CP_BQ8 + ft),
                                            op0=ALU.mult, op1=ALU.add)
                    o2 = qwork.tile([128, TQ], F32, tag="o2")
                    nc.vector.tensor_mul(o2, pss[f2], lnB["a"][0])
                    q = p1.tile([128, TQ], F32R, tag=f"qT{ft}")
                    nc.vector.tensor_add(q, o1, o2)
                    qT.append(q)

        a_bcs, c_bcs = lnA["a"], lnA["c"]
        a_col, c_col = lnA["acol"], lnA["ccol"]

        # ------------- Phases 3-4: K/V + attention, two half passes -------------
        with ExitStack() as stc:
            ypool = stc.enter_context(tc.tile_pool(name="ypool", bufs=1))
            yT = []
            for ft in range(NCT):
                yt_tile = ypool.tile([128, TQ], F32, tag=f"yT{ft}")
                yT.append(yt_tile)

            for half in range(2):
                with ExitStack() as sth:
                    vpool = sth.enter_context(tc.tile_pool(name=f"v{half}", bufs=1))
                    v_sb = []
                    with ExitStack() as st:
                        wvp = st.enter_context(tc.tile_pool(name=f"wv{half}", bufs=1))
                        vps = st.enter_context(
                            tc.tile_pool(name=f"vps{half}", bufs=3, space="PSUM"))
                        wv_tiles = []
                        for ct in range(NCT):
                            wt = wvp.tile([128, 512], F32R, tag=f"wv{ct}")
                            nc.sync.dma_start(
                                out=wt,
                                in_=wv.ap()[ct * 128:(ct + 1) * 128,
                                            half * 512:half * 512 + 512].bitcast(F32R))
                            wv_tiles.append(wt)
                        for tt in range(NTT):
                            v = vpool.tile([128, 8, 65], F32R, tag=f"v{tt}")
                            nc.gpsimd.memset(v[:, :, 64:65].bitcast(F32), 1.0)
                            v_sb.append(v)
                            ps = vps.tile([128, 512], F32, tag="ps")
                            for ct in range(NCT):
                                nc.tensor.matmul(
                                    ps,
                                    xT[ct][tt // 4][:, (tt % 4) * 128:(tt % 4) * 128 + 128],
                                    wv_tiles[ct],
                                    start=(ct == 0), stop=(ct == NCT - 1))
                            o1 = wvp.tile([128, 512], F32, tag="vo1")
                            nc.vector.tensor_scalar_mul(
                                o1, rw_bc[:, half * 512:half * 512 + 512],
                                c_col[:, tt:tt + 1])
                            o2 = wvp.tile([128, 512], F32, tag="vo2")
                            nc.vector.tensor_scalar_mul(o2, ps, a_col[:, tt:tt + 1])
                            nc.vector.tensor_add(
                                v[:, :, 0:64],
                                o2.rearrange("p (h d) -> p h d", h=8),
                                o1.rearrange("p (h d) -> p h d", h=8))

                    with ExitStack() as st:
                        kps = st.enter_context(
                            tc.tile_pool(name=f"kps{half}", bufs=1, space="PSUM"))
                        sps = st.enter_context(
                            tc.tile_pool(name=f"sps{half}", bufs=2, space="PSUM"))
                        bps = st.enter_context(
                            tc.tile_pool(name=f"bps{half}", bufs=1, space="PSUM"))
                        ops_ = st.enter_context(
                            tc.tile_pool(name=f"ops{half}", bufs=1, space="PSUM"))
                        kpool = st.enter_context(tc.tile_pool(name=f"k{half}", bufs=1))
                        wkp = st.enter_context(tc.tile_pool(name=f"wk{half}", bufs=1))
                        epool = st.enter_context(tc.tile_pool(name=f"e{half}", bufs=2))
                        fpool = st.enter_context(tc.tile_pool(name=f"f{half}", bufs=1))
                        rpool = st.enter_context(tc.tile_pool(name=f"r{half}", bufs=2))

                        def kproj_gen(hp, side):
                            """Emit K-projection for pair hp as resumable steps."""
                            wk_tiles = []
                            for ct in range(NCT):
                                wt = wkp.tile([128, 128], F32R, tag=f"w{side}{ct}")
                                nc.sync.dma_start(
                                    out=wt, in_=wk_g.ap()[ct, hp].bitcast(F32R))
                                wk_tiles.append(wt)
                            chunks = []
                            for ch in range(4):
                                kc = kpool.tile([128, 512], F32R, tag=f"k{side}{ch}")
                                chunks.append(kc)

                            def steps():
                                for ch in range(4):
                                    ps = kps.tile([128, 512], F32, tag="ps")
                                    for ct in range(NCT):
                                        nc.tensor.matmul(
                                            ps, wk_tiles[ct], xT[ct][ch],
                                            start=(ct == 0), stop=(ct == NCT - 1))
                                        yield
                                    o1 = wkp.tile([128, 512], F32, tag="ko1")
                                    nc.vector.tensor_scalar(
                                        out=o1, in0=c_bcs[ch],
                                        scalar1=col(CP_CWK + hp),
                                        scalar2=col(CP_BK + hp),
                                        op0=ALU.mult, op1=ALU.add)
                                    yield
                                    o2 = wkp.tile([128, 512], F32, tag="ko2")
                                    nc.vector.tensor_mul(o2, ps, a_bcs[ch])
                                    yield
                                    nc.vector.tensor_add(chunks[ch], o1, o2)
                                    yield
                            return chunks, steps()

                        def emit_scores(hp, cur_chunks, kt):
                            exs = []
                            for hh in range(2):
                                p0 = 64 * hh
                                sc = sps.tile([128, 512], F32, tag=f"sc{hh}")
                                nc.tensor.matmul(
                                    sc,
                                    cur_chunks[kt // 4][p0:p0 + 64,
                                                        (kt % 4) * 128:(kt % 4) * 128 + 128],
                                    qT[hp][p0:p0 + 64, :], start=True, stop=True,
                                    tile_position=(p0, 0))
                                ex = epool.tile([128, 512], F32R, tag=f"ex{hh}")
                                nc.scalar.activation(ex, sc, ACTF.Exp)
                                exs.append(ex)
                            return exs

                        cur_chunks, gen = kproj_gen(half * 4, "A")
                        for _ in gen:
                            pass
                        for hp_local in range(4):
                            hp = half * 4 + hp_local
                            side = "AB"[hp_local % 2]
                            nside = "AB"[(hp_local + 1) % 2]
                            if hp_local + 1 < 4:
                                next_chunks, next_gen = kproj_gen(hp + 1, nside)
                            else:
                                next_chunks, next_gen = None, iter(())
                            out_AB = []
                            for hh in range(2):
                                o = ops_.tile([65, 512], F32, tag=f"out{hh}")
                                out_AB.append(o)
                            prev_exs = None
                            for kt in range(NTT):
                                exs = emit_scores(hp, cur_chunks, kt)
                                for _ in range(3):
                                    next(next_gen, None)
                                if prev_exs is not None:
                                    for hh in range(2):
                                        h = hp * 2 + hh
                                        nc.tensor.matmul(
                                            out_AB[hh], v_sb[kt - 1][:, h % 8, :],
                                            prev_exs[hh],
                                            start=(kt - 1 == 0), stop=False)
                                prev_exs = exs
                            for hh in range(2):
                                h = hp * 2 + hh
                                nc.tensor.matmul(
                                    out_AB[hh], v_sb[NTT - 1][:, h % 8, :],
                                    prev_exs[hh], start=False, stop=True)
                            for _ in next_gen:
                                pass
                            for hh in range(2):
                                p0 = 64 * hh
                                out_ps = out_AB[hh]
                                rr0 = rpool.tile([1, 512], F32, tag="rr0")
                                nc.vector.tensor_copy(rr0, out_ps[64:65, :])
                                rr = rpool.tile([1, 512], F32, tag="rr")
                                nc.vector.reciprocal_approx_fast(out=rr, in_=rr0)
                                rr_r = rpool.tile([1, 512], F32R, tag="rr_r")
                                nc.scalar.activation(rr_r, rr, ACTF.Copy)
                                bc = bps.tile([64, 512], F32, tag="bc")
                                nc.tensor.matmul(bc, ones_row[:, 0:64], rr_r,
                                                 start=True, stop=True)
                                bc_sb = fpool.tile([64, 512], F32, tag="bcs")
                                nc.vector.tensor_copy(bc_sb, bc)
                                t1 = fpool.tile([64, 512], F32, tag="yt")
                                nc.vector.tensor_mul(t1, out_ps[0:64, :], bc_sb)
                                nc.vector.tensor_scalar_add(
                                    yT[hp][p0:p0 + 64, :], t1,
                                    col(CP_BV + hp)[p0:p0 + 64, :])
                            cur_chunks = next_chunks

            # -------- Phase 5: attention out proj + residual --------
            with ExitStack() as st:
                xrp = st.enter_context(tc.tile_pool(name="xrp", bufs=1))
                wpool = st.enter_context(tc.tile_pool(name="wo", bufs=3))
                pps = st.enter_context(tc.tile_pool(name="ops2", bufs=1, space="PSUM"))
                tpool = st.enter_context(tc.tile_pool(name="t5", bufs=2))
                yT_r = []
                for ft in range(NCT):
                    r = xrp.tile([128, TQ], F32R, tag=f"yTr{ft}")
                    nc.scalar.activation(r, yT[ft], ACTF.Copy)
                    yT_r.append(r)
                xq_res = []
                for ct in range(NCT):
                    xr = xrp.tile([128, TQ], F32, tag=f"xqres{ct}")
                    nc.sync.dma_start(out=xr,
                                      in_=xqT.ap()[ct * 128:(ct + 1) * 128, :])
                    xq_res.append(xr)
                for grp in range(2):
                    pss = []
                    for f4 in range(4):
                        ps = pps.tile([128, TQ], F32, tag=f"ps{f4}")
                        pss.append(ps)
                    for ct in range(NCT):
                        wt = wpool.tile([128, 512], F32R, tag="w")
                        nc.sync.dma_start(out=wt, in_=wo_p.ap()[ct, grp].bitcast(F32R))
                        for f4 in range(4):
                            nc.tensor.matmul(pss[f4], wt[:, f4 * 128:f4 * 128 + 128],
                                             yT_r[ct], start=(ct == 0),
                                             stop=(ct == NCT - 1))
                    for f4 in range(4):
                        ft = grp * 4 + f4
                        t = tpool.tile([128, TQ], F32, tag="t")
                        nc.vector.tensor_add(t, pss[f4], xq_res[ft])
                        x2 = tpool.tile([128, TQ], F32, tag="x2")
                        nc.scalar.activation(x2, t, ACTF.Identity,
                                             bias=col(CP_BO + ft))
                        nc.sync.dma_start(
                            out=x2_d.ap()[ft * 128:(ft + 1) * 128, :], in_=x2)

        sp1.close()  # free raw-x/qT/yT region before the MLP phases

        # ---------------- Phase 6-8: LN2 + MLP ----------------
        gpool = top.enter_context(tc.tile_pool(name="gpool", bufs=1))
        x2p = top.enter_context(tc.tile_pool(name="x2p", bufs=1))
        x2T = []
        for ct in range(NCT):
            x2t_t = x2p.tile([128, TQ], F32R, tag=f"x2L{ct}")
            nc.sync.dma_start(
                out=x2t_t, in_=x2_d.ap()[ct * 128:(ct + 1) * 128, :].bitcast(F32R))
            x2T.append(x2t_t)
        with ExitStack() as stg:
            hpool = stg.enter_context(tc.tile_pool(name="hpool", bufs=1))
            with ExitStack() as st:
                pools2 = make_ln_pools(st, "ln2")
                abc2 = ln_stats(x2T, TQ, "H", pools2, pools2["bc"])
                hT = ln_apply(x2T, TQ, "H", CP_G2, CP_BL2, hpool, "hT",
                              pools2, abc2)
            gT = []
            with ExitStack() as st:
                wpool = st.enter_context(tc.tile_pool(name="w1", bufs=4))
                pps = st.enter_context(tc.tile_pool(name="m1ps", bufs=1, space="PSUM"))
                for grp in range(8):
                    pss = []
                    for f4 in range(4):
                        ps = pps.tile([128, TQ], F32, tag=f"ps{f4}")
                        pss.append(ps)
                    for ct in range(NCT):
                        wt = wpool.tile([128, 512], F32R, tag="w")
                        nc.sync.dma_start(out=wt, in_=w1_p.ap()[ct, grp].bitcast(F32R))
                        for f4 in range(4):
                            nc.tensor.matmul(pss[f4], wt[:, f4 * 128:f4 * 128 + 128],
                                             hT[ct][0], start=(ct == 0),
                                             stop=(ct == NCT - 1))
                    for f4 in range(4):
                        hf = grp * 4 + f4
                        g = gpool.tile([128, TQ], F32R, tag=f"gT{hf}")
                        nc.scalar.activation(g, pss[f4], ACTF.Gelu,
                                             bias=col(CP_B1 + hf))
                        gT.append(g)

        with ExitStack() as st:
            wpool = st.enter_context(tc.tile_pool(name="w2", bufs=4))
            pps = st.enter_context(tc.tile_pool(name="m2ps", bufs=1, space="PSUM"))
            tpool = st.enter_context(tc.tile_pool(name="t8", bufs=3))
            for grp in range(2):
                pss = []
                for f4 in range(4):
                    ps = pps.tile([128, TQ], F32, tag=f"ps{f4}")
                    pss.append(ps)
                for hf in range(NHT):
                    wt = wpool.tile([128, 512], F32R, tag="w")
                    nc.sync.dma_start(out=wt, in_=w2_p.ap()[hf, grp].bitcast(F32R))
                    for f4 in range(4):
                        nc.tensor.matmul(pss[f4], wt[:, f4 * 128:f4 * 128 + 128],
                                         gT[hf], start=(hf == 0),
                                         stop=(hf == NHT - 1))
                for f4 in range(4):
                    ft = grp * 4 + f4
                    t = tpool.tile([128, TQ], F32, tag="t")
                    nc.scalar.activation(t, pss[f4], ACTF.Identity,
                                         bias=col(CP_B2 + ft))
                    o = tpool.tile([128, TQ], F32, tag="o")
                    nc.vector.tensor_add(o, t, x2T[ft].bitcast(F32))
                    nc.sync.dma_start(out=out_d.ap()[ft * 128:(ft + 1) * 128, :],
                                      in_=o)

    nc.compile()
    return nc


def _prep_inputs(inputs):
    f64 = np.float64
    x = np.asarray(inputs["x"], np.float32)
    g1 = np.asarray(inputs["ln1_g"], f64)
    b1v = np.asarray(inputs["ln1_b"], f64)
    Wq = np.asarray(inputs["Wq"], f64) * g1[:, None]
    Wk = np.asarray(inputs["Wk"], f64) * g1[:, None]
    Wv = np.asarray(inputs["Wv"], f64) * g1[:, None]
    bq8_eff = 0.125 * (b1v @ np.asarray(inputs["Wq"], f64)
                       + np.asarray(inputs["bq"], f64))
    bk_eff = b1v @ np.asarray(inputs["Wk"], f64) + np.asarray(inputs["bk"], f64)
    bv_eff = b1v @ np.asarray(inputs["Wv"], f64) + np.asarray(inputs["bv"], f64)
    colWq8 = 0.125 * Wq.sum(0)
    colWk = Wk.sum(0)
    rowWv = Wv.sum(0)

    common = dict(
        wq_p=_grid2((0.125 * Wq).astype(np.float32)),
        wk_g=_grid(Wk.astype(np.float32)),
        wv=np.ascontiguousarray(Wv.astype(np.float32)),
        wo_p=_grid4(np.asarray(inputs["Wo"], np.float32)),
        w1_p=_grid4(np.asarray(inputs["W1"], np.float32)),
        w2_p=_grid4(np.asarray(inputs["W2"], np.float32)),
        rowwv=rowWv.astype(np.float32),
    )
    cpk = np.zeros((128, CP_N), np.float32)
    cpk[:, CP_BQ8:CP_BQ8 + 8] = _pack_cols(bq8_eff)
    cpk[:, CP_BK:CP_BK + 8] = _pack_cols(bk_eff)
    cpk[:, CP_BO:CP_BO + 8] = _pack_cols(np.asarray(inputs["bo"], np.float32))
    cpk[:, CP_B2:CP_B2 + 8] = _pack_cols(np.asarray(inputs["b2"], np.float32))
    cpk[:, CP_G2:CP_G2 + 8] = _pack_cols(np.asarray(inputs["ln2_g"], np.float32))
    cpk[:, CP_BL2:CP_BL2 + 8] = _pack_cols(np.asarray(inputs["ln2_b"], np.float32))
    cpk[:, CP_BV:CP_BV + 8] = _pack_cols(bv_eff)
    cpk[:, CP_B1:CP_B1 + 32] = _pack_cols(np.asarray(inputs["b1"], np.float32))
    cpk[:, CP_EPS] = LN_EPS
    cpk[:, CP_NEG1] = -1.0
    cpk[:, CP_QSCL] = 0.125
    cpk[:, CP_CWK:CP_CWK + 8] = _pack_cols(colWk)
    cpk[:, CP_CWQ8:CP_CWQ8 + 8] = _pack_cols(colWq8)
    common["colpack"] = cpk

    in_maps = []
    for core in range(N_CORES):
        b, s = divmod(core, N_CORES // B)
        m = dict(common)
        m["xbT"] = np.ascontiguousarray(x[b].T)
        m["xqT"] = np.ascontiguousarray(x[b, s * TQ:(s + 1) * TQ, :].T)
        in_maps.append(m)
    return in_maps


def kernel(**inputs):
    from concourse.bass_utils import run_bass_kernel_spmd
    if "nc" not in _CACHE:
        _CACHE["nc"] = _build_program()
    nc = _CACHE["nc"]
    in_maps = _prep_inputs(inputs)
    res = run_bass_kernel_spmd(nc, in_maps, list(range(N_CORES)))
    out = np.empty((B, T, C), np.float32)
    for core in range(N_CORES):
        b, s = divmod(core, N_CORES // B)
        out[b, s * TQ:(s + 1) * TQ, :] = res.results[core]["outT"].T
    return out



# revision 2
# speedup vs baseline: 96.1026x; 96.1026x over previous
"""Trainium2 Bass kernel for a dense transformer block (nn_Block_7911329760080).

Reference computation (B=2, T=2048 tokens, C=1024 channels, 16 heads, fp32):
    x = x + Attn(LN1(x));  x = x + MLP(LN2(x))   [full non-causal attention]

Sharding: token-parallel over 8 cores (4 cores per batch, 512 tokens each).
Each core recomputes LN1 stats + K + V for its whole batch (no collectives);
Q / attention / MLP only for its own 512-token slice.

Layout: activations are feature-major ([feature, token], "T" suffix) so every
matmul (out = lhsT.T @ rhs, contraction on partitions) directly produces the
layout the next one consumes — zero on-device transposes.

LN1 is FOLDED into the Q/K/V projections: LN(x) = a_t * x + c_t per token
(with gamma/beta absorbed into the weights on the host), and since the
projections are linear, proj(LN(x)) = a_t * proj(x) + c_t * colsum(W') + bias.
The projections therefore run on RAW x (available immediately), and the tiny
rank-1 fixup happens during PSUM evacuation.  LN statistics come from
ones-vector matmuls; softmax is max-free (scores are small) with the per-query
normalizer produced free by an interleaved ones-column in V during the P@V
matmul.  All matmuls run in float32r (full PE rate at moving dim >= 256).
The attention inner loop is software-pipelined: scores run one step ahead of
P@V, and the next head-pair's K-projection matmuls fill the exp-wait slack.
"""

import numpy as np
import sys
from contextlib import ExitStack

sys.path.insert(0, "/opt/trn_rl_repo/concourse")
sys.path.insert(0, "/opt/trn_rl_repo")

import concourse.bass as bass
import concourse.bacc as bacc
import concourse.mybir as mybir
import concourse.tile as tile

F32 = mybir.dt.float32
F32R = mybir.dt.float32r
ACTF = mybir.ActivationFunctionType
ALU = mybir.AluOpType

N_CORES = 8
B, T, C = 2, 2048, 1024
NH, HD = 16, 64
TQ = T * B // N_CORES          # 512 tokens per core
HID = 4 * C                    # 4096
NCT = C // 128                 # 8 c-tiles
NHT = HID // 128               # 32 hidden-dim tiles
NTT = T // 128                 # 16 token-tiles (full batch)
NCH = T // 512                 # 4 chunks (a chunk = 512 tokens)
LN_EPS = 1e-5

# colpack column layout ([128, n] per-partition bias/scale columns)
CP_BQ8, CP_BK, CP_BO, CP_B2, CP_G1, CP_BL1, CP_G2, CP_BL2, CP_BV = (
    0, 8, 16, 24, 32, 40, 48, 56, 64)
CP_B1 = 72          # 32 cols
CP_EPS, CP_NEG1, CP_QSCL = 104, 105, 106
CP_CWK, CP_CWQ8 = 107, 115
CP_N = 123

_CACHE = {}


def _pack_cols(vec):
    """[n*128] -> [128, n]: column j holds vec[128j:128j+128]."""
    return np.ascontiguousarray(vec.astype(np.float32).reshape(-1, 128).T)


def _grid(w):
    K, F = w.shape
    return np.ascontiguousarray(
        w.reshape(K // 128, 128, F // 128, 128).transpose(0, 2, 1, 3))


def _grid4(w):
    K, F = w.shape
    return np.ascontiguousarray(
        w.reshape(K // 128, 128, F // 512, 512).transpose(0, 2, 1, 3))


def _grid2(w):
    K, F = w.shape
    return np.ascontiguousarray(
        w.reshape(K // 128, 128, F // 256, 256).transpose(0, 2, 1, 3))


def _build_program():
    nc = bacc.Bacc("TRN2", target_bir_lowering=False, debug=False,
                   num_devices=N_CORES)

    def din(name, shape):
        return nc.dram_tensor(name, list(shape), F32, kind="ExternalInput")

    xbT = din("xbT", (C, T))            # full batch, feature-major (raw)
    xqT = din("xqT", (C, TQ))           # own token slice, feature-major (raw)
    wq_p = din("wq_p", (NCT, 4, 128, 256))
    wk_g = din("wk_g", (NCT, NCT, 128, 128))
    wv = din("wv", (C, C))              # gamma-scaled, natural layout
    wo_p = din("wo_p", (NCT, 2, 128, 512))
    w1_p = din("w1_p", (NCT, 8, 128, 512))
    w2_p = din("w2_p", (NHT, 2, 128, 512))
    rowwv = din("rowwv", (C,))          # colsum of gamma-scaled Wv
    colpack = din("colpack", (128, CP_N))
    out_d = nc.dram_tensor("outT", [C, TQ], F32, kind="ExternalOutput")
    acr_d = nc.dram_tensor("acr", [2, T], F32)   # bounce for a/c scatter
    x2_d = nc.dram_tensor("x2d", [C, TQ], F32)   # x + attn(LN1(x)), spilled

    with tile.TileContext(nc) as tc, ExitStack() as top:
        consts = top.enter_context(tc.tile_pool(name="consts", bufs=1))

        cp = consts.tile([128, CP_N], F32)
        nc.sync.dma_start(out=cp, in_=colpack.ap())
        ones_col = consts.tile([128, 1], F32R)
        nc.vector.memset(ones_col.bitcast(F32), 1.0)
        ones_row = consts.tile([1, 128], F32R)
        nc.vector.memset(ones_row.bitcast(F32), 1.0)
        rw_bc = consts.tile([128, C], F32)
        rw_src = rowwv.ap()
        rw_src = bass.AP(tensor=rw_src.tensor, offset=rw_src.offset,
                         ap=[[0, 128]] + list(rw_src.ap))
        nc.sync.dma_start(out=rw_bc, in_=rw_src)

        def col(idx):
            return cp[:, idx:idx + 1]

        def row_const(idx):
            return cp[0:1, idx:idx + 1]

        # ---------------- LayerNorm stats (feature-major) ----------------
        def make_ln_pools(st, name, psum=True):
            pools = {}
            if psum:
                pools["ps_st"] = st.enter_context(
                    tc.tile_pool(name=f"{name}_ps", bufs=2, space="PSUM"))
                pools["ps_bc"] = st.enter_context(
                    tc.tile_pool(name=f"{name}_pb", bufs=1, space="PSUM"))
            pools["work"] = st.enter_context(tc.tile_pool(name=f"{name}_w", bufs=3))
            pools["rows"] = st.enter_context(tc.tile_pool(name=f"{name}_r", bufs=1))
            pools["bc"] = st.enter_context(tc.tile_pool(name=f"{name}_b", bufs=1))
            return pools

        def src_tile(src, ct, ch):
            e = src[ct]
            return e[ch] if isinstance(e, list) else e

        def ln_stats(src, Tn, tag, pools, bc_pool, scale8=False, cols=False):
            """src: [ct] or [ct][ch] SBUF tiles [128,512] F32R (raw x).
            Computes LN(x) = a_t*x + c_t.  Returns dict with per-chunk
            broadcast tiles 'a' (or 'a8' = a/8), 'c', and optionally
            token-major column tiles 'acol'/'ccol' [128, Tn/128]."""
            nch = Tn // 512
            rows, work = pools["rows"], pools["work"]
            res = {"a": [], "c": []}
            arow = crow = None
            if cols:
                arow = rows.tile([1, Tn], F32, tag=f"{tag}arow")
                crow = rows.tile([1, Tn], F32, tag=f"{tag}crow")
            for ch in range(nch):
                sl = slice(ch * 512, ch * 512 + 512)
                ps_s = pools["ps_st"].tile([1, 512], F32, tag="ps_s")
                ps_q = pools["ps_st"].tile([1, 512], F32, tag="ps_q")
                xcs = [src_tile(src, ct, ch) for ct in range(NCT)]
                sqs = []
                for ct in range(NCT):
                    sq = work.tile([128, 512], F32R, tag="sq")
                    nc.vector.tensor_mul(sq, xcs[ct].bitcast(F32),
                                         xcs[ct].bitcast(F32))
                    sqs.append(sq)
                for ct in range(NCT):
                    nc.tensor.matmul(ps_s, ones_col, xcs[ct],
                                     start=(ct == 0), stop=(ct == NCT - 1))
                for ct in range(NCT):
                    nc.tensor.matmul(ps_q, ones_col, sqs[ct],
                                     start=(ct == 0), stop=(ct == NCT - 1))
                mu = rows.tile([1, 512], F32, tag=f"{tag}mu")
                nc.vector.tensor_scalar_mul(mu, ps_s, 1.0 / C)
                msq = rows.tile([1, 512], F32, tag=f"{tag}msq")
                nc.vector.tensor_scalar_mul(msq, ps_q, 1.0 / C)
                mu2 = rows.tile([1, 512], F32, tag=f"{tag}mu2")
                nc.vector.tensor_mul(mu2, mu, mu)
                nc.vector.tensor_sub(msq, msq, mu2)
                std = rows.tile([1, 512], F32, tag=f"{tag}std")
                nc.scalar.activation(std, msq, ACTF.Sqrt, bias=row_const(CP_EPS))
                nc.vector.reciprocal_approx_fast(out=std, in_=std)  # std <- rstd
                nc.vector.tensor_mul(mu, mu, std)                   # mu <- mu*rstd
                if cols:
                    nc.vector.tensor_copy(arow[:, sl], std)
                    nc.vector.tensor_scalar_mul(crow[:, sl], mu, -1.0)
                if scale8:
                    nc.vector.tensor_scalar_mul(std, std, 0.125)
                rstd_r = rows.tile([1, 512], F32R, tag=f"{tag}rr")
                nc.scalar.activation(rstd_r, std, ACTF.Copy)
                nmu_r = rows.tile([1, 512], F32R, tag=f"{tag}nm")
                nc.scalar.activation(nmu_r, mu, ACTF.Copy, scale=row_const(CP_NEG1))
                ps_a = pools["ps_bc"].tile([128, 512], F32, tag="ps_a")
                nc.tensor.matmul(ps_a, ones_row, rstd_r, start=True, stop=True)
                a_bc = bc_pool.tile([128, 512], F32, tag=f"{tag}a{ch}")
                nc.vector.tensor_copy(a_bc, ps_a)
                res["a"].append(a_bc)
                ps_c = pools["ps_bc"].tile([128, 512], F32, tag="ps_c")
                nc.tensor.matmul(ps_c, ones_row, nmu_r, start=True, stop=True)
                c_bc = bc_pool.tile([128, 512], F32, tag=f"{tag}c{ch}")
                nc.vector.tensor_copy(c_bc, ps_c)
                res["c"].append(c_bc)
            if cols:
                # scatter [1,Tn] rows to token-major [128, Tn/128] columns
                # via a DRAM bounce (free-axis -> partition-axis move)
                nc.sync.dma_start(out=acr_d.ap()[0, 0:Tn], in_=arow)
                nc.sync.dma_start(out=acr_d.ap()[1, 0:Tn], in_=crow)
                acol = bc_pool.tile([128, Tn // 128], F32, tag=f"{tag}acol")
                nc.sync.dma_start(
                    out=acol,
                    in_=acr_d.ap()[0, 0:Tn].rearrange("(tt p) -> p tt", p=128))
                ccol = bc_pool.tile([128, Tn // 128], F32, tag=f"{tag}ccol")
                nc.sync.dma_start(
                    out=ccol,
                    in_=acr_d.ap()[1, 0:Tn].rearrange("(tt p) -> p tt", p=128))
                res["acol"], res["ccol"] = acol, ccol
            return res

        def ln_apply(src, Tn, tag, g_idx, b_idx, out_pool, out_tag, pools, abc):
            a_bcs, c_bcs = abc["a"], abc["c"]
            nch = Tn // 512
            work = pools["work"]
            outs = [[None] * nch for _ in range(NCT)]
            for ch in range(nch):
                for ct in range(NCT):
                    o = out_pool.tile([128, 512], F32R, tag=f"{out_tag}{ct}_{ch}")
                    xc = src_tile(src, ct, ch)
                    t1 = work.tile([128, 512], F32, tag="t1")
                    nc.vector.tensor_mul(t1, xc.bitcast(F32), a_bcs[ch])
                    t2 = work.tile([128, 512], F32, tag="t2")
                    nc.vector.tensor_add(t2, t1, c_bcs[ch])
                    nc.scalar.activation(o, t2, ACTF.Identity,
                                         scale=col(g_idx + ct), bias=col(b_idx + ct))
                    outs[ct][ch] = o
            return outs

        # persistent pool: raw x tiles, qT, LN1 fixup tiles (to attention end)
        sp1 = top.enter_context(ExitStack())
        p1 = sp1.enter_context(tc.tile_pool(name="p1", bufs=1))

        # raw activations, feature-major, chunked [128,512]
        xT = []
        for ct in range(NCT):
            per_ct = []
            for ch in range(NCH):
                xt_c = p1.tile([128, 512], F32R, tag=f"xT{ct}_{ch}")
                nc.sync.dma_start(
                    out=xt_c,
                    in_=xbT.ap()[ct * 128:(ct + 1) * 128,
                                 ch * 512:ch * 512 + 512].bitcast(F32R))
                per_ct.append(xt_c)
            xT.append(per_ct)
        # -------- Phases 1-2: LN stats and folded Q projection --------
        qT = []
        with ExitStack() as stln:
            xqp = stln.enter_context(tc.tile_pool(name="xqp", bufs=1))
            xq_raw = []
            for ct in range(NCT):
                xt_q = xqp.tile([128, 512], F32R, tag=f"xq{ct}")
                nc.sync.dma_start(
                    out=xt_q, in_=xqT.ap()[ct * 128:(ct + 1) * 128, :].bitcast(F32R))
                xq_raw.append(xt_q)
            pools = make_ln_pools(stln, "ln0")
            lnB = ln_stats(xq_raw, TQ, "B", pools, pools["bc"], scale8=True)
            lnA = ln_stats(xT, T, "A", pools, p1, cols=True)
            # Q projection on raw xq; LN fold + bias + 1/8 scale in the evac
            wpool = stln.enter_context(tc.tile_pool(name="wq", bufs=3))
            qps = stln.enter_context(tc.tile_pool(name="qps", bufs=1, space="PSUM"))
            qwork = stln.enter_context(tc.tile_pool(name="qw", bufs=2))
            for grp in range(4):
                pss = []
                for f2 in range(2):
                    ps = qps.tile([128, TQ], F32, tag=f"ps{f2}")
                    pss.append(ps)
                for ct in range(NCT):
                    wt = wpool.tile([128, 256], F32R, tag="w")
                    nc.sync.dma_start(out=wt, in_=wq_p.ap()[ct, grp].bitcast(F32R))
                    for f2 in range(2):
                        nc.tensor.matmul(pss[f2], wt[:, f2 * 128:f2 * 128 + 128],
                                         xq_raw[ct], start=(ct == 0),
                                         stop=(ct == NCT - 1))
                for f2 in range(2):
                    ft = grp * 2 + f2
                    o1 = qwork.tile([128, TQ], F32, tag="o1")
                    nc.vector.tensor_scalar(out=o1, in0=lnB["c"][0],
                                            scalar1=col(CP_CWQ8 + ft),
                                            scalar2=col(CP_BQ8 + ft),
                                            op0=ALU.mult, op1=ALU.add)
                    o2 = qwork.tile([128, TQ], F32, tag="o2")
                    nc.vector.tensor_mul(o2, pss[f2], lnB["a"][0])
                    q = p1.tile([128, TQ], F32R, tag=f"qT{ft}")
                    nc.vector.tensor_add(q, o1, o2)
                    qT.append(q)

        a_bcs, c_bcs = lnA["a"], lnA["c"]
        a_col, c_col = lnA["acol"], lnA["ccol"]

        # ------------- Phases 3-4: K/V + attention, two half passes -------------
        with ExitStack() as stc:
            ypool = stc.enter_context(tc.tile_pool(name="ypool", bufs=1))
            yT = []
            for ft in range(NCT):
                yt_tile = ypool.tile([128, TQ], F32, tag=f"yT{ft}")
                yT.append(yt_tile)

            for half in range(2):
                with ExitStack() as sth:
                    vpool = sth.enter_context(tc.tile_pool(name=f"v{half}", bufs=1))
                    v_sb = []
                    with ExitStack() as st:
                        wvp = st.enter_context(tc.tile_pool(name=f"wv{half}", bufs=1))
                        vps = st.enter_context(
                            tc.tile_pool(name=f"vps{half}", bufs=3, space="PSUM"))
                        wv_tiles = []
                        for ct in range(NCT):
                            wt = wvp.tile([128, 512], F32R, tag=f"wv{ct}")
                            nc.sync.dma_start(
                                out=wt,
                                in_=wv.ap()[ct * 128:(ct + 1) * 128,
                                            half * 512:half * 512 + 512].bitcast(F32R))
                            wv_tiles.append(wt)
                        for tt in range(NTT):
                            v = vpool.tile([128, 8, 65], F32R, tag=f"v{tt}")
                            nc.gpsimd.memset(v[:, :, 64:65].bitcast(F32), 1.0)
                            v_sb.append(v)
                            ps = vps.tile([128, 512], F32, tag="ps")
                            for ct in range(NCT):
                                nc.tensor.matmul(
                                    ps,
                                    xT[ct][tt // 4][:, (tt % 4) * 128:(tt % 4) * 128 + 128],
                                    wv_tiles[ct],
                                    start=(ct == 0), stop=(ct == NCT - 1))
                            o1 = wvp.tile([128, 512], F32, tag="vo1")
                            nc.vector.tensor_scalar_mul(
                                o1, rw_bc[:, half * 512:half * 512 + 512],
                                c_col[:, tt:tt + 1])
                            o2 = wvp.tile([128, 512], F32, tag="vo2")
                            nc.vector.tensor_scalar_mul(o2, ps, a_col[:, tt:tt + 1])
                            nc.vector.tensor_add(
                                v[:, :, 0:64],
                                o2.rearrange("p (h d) -> p h d", h=8),
                                o1.rearrange("p (h d) -> p h d", h=8))

                    with ExitStack() as st:
                        kps = st.enter_context(
                            tc.tile_pool(name=f"kps{half}", bufs=1, space="PSUM"))
                        sps = st.enter_context(
                            tc.tile_pool(name=f"sps{half}", bufs=2, space="PSUM"))
                        bps = st.enter_context(
                            tc.tile_pool(name=f"bps{half}", bufs=1, space="PSUM"))
                        ops_ = st.enter_context(
                            tc.tile_pool(name=f"ops{half}", bufs=1, space="PSUM"))
                        kpool = st.enter_context(tc.tile_pool(name=f"k{half}", bufs=1))
                        wkp = st.enter_context(tc.tile_pool(name=f"wk{half}", bufs=1))
                        epool = st.enter_context(tc.tile_pool(name=f"e{half}", bufs=2))
                        fpool = st.enter_context(tc.tile_pool(name=f"f{half}", bufs=1))
                        rpool = st.enter_context(tc.tile_pool(name=f"r{half}", bufs=2))

                        def kproj_gen(hp, side):
                            """Emit K-projection for pair hp as resumable steps."""
                            wk_tiles = []
                            for ct in range(NCT):
                                wt = wkp.tile([128, 128], F32R, tag=f"w{side}{ct}")
                                nc.sync.dma_start(
                                    out=wt, in_=wk_g.ap()[ct, hp].bitcast(F32R))
                                wk_tiles.append(wt)
                            chunks = []
                            for ch in range(4):
                                kc = kpool.tile([128, 512], F32R, tag=f"k{side}{ch}")
                                chunks.append(kc)

                            def steps():
                                for ch in range(4):
                                    ps = kps.tile([128, 512], F32, tag="ps")
                                    for ct in range(NCT):
                                        nc.tensor.matmul(
                                            ps, wk_tiles[ct], xT[ct][ch],
                                            start=(ct == 0), stop=(ct == NCT - 1))
                                        yield
                                    o1 = wkp.tile([128, 512], F32, tag="ko1")
                                    nc.vector.tensor_scalar(
                                        out=o1, in0=c_bcs[ch],
                                        scalar1=col(CP_CWK + hp),
                                        scalar2=col(CP_BK + hp),
                                        op0=ALU.mult, op1=ALU.add)
                                    yield
                                    o2 = wkp.tile([128, 512], F32, tag="ko2")
                                    nc.vector.tensor_mul(o2, ps, a_bcs[ch])
                                    yield
                                    nc.vector.tensor_add(chunks[ch], o1, o2)
                                    yield
                            return chunks, steps()

                        def emit_scores(hp, cur_chunks, kt):
                            exs = []
                            for hh in range(2):
                                p0 = 64 * hh
                                sc = sps.tile([128, 512], F32, tag=f"sc{hh}")
                                nc.tensor.matmul(
                                    sc,
                                    cur_chunks[kt // 4][p0:p0 + 64,
                                                        (kt % 4) * 128:(kt % 4) * 128 + 128],
                                    qT[hp][p0:p0 + 64, :], start=True, stop=True,
                                    tile_position=(p0, 0))
                                ex = epool.tile([128, 512], F32R, tag=f"ex{hh}")
                                nc.scalar.activation(ex, sc, ACTF.Exp)
                                exs.append(ex)
                            return exs

                        cur_chunks, gen = kproj_gen(half * 4, "A")
                        for _ in gen:
                            pass
                        for hp_local in range(4):
                            hp = half * 4 + hp_local
                            side = "AB"[hp_local % 2]
                            nside = "AB"[(hp_local + 1) % 2]
                            if hp_local + 1 < 4:
                                next_chunks, next_gen = kproj_gen(hp + 1, nside)
                            else:
                                next_chunks, next_gen = None, iter(())
                            out_AB = []
                            for hh in range(2):
                                o = ops_.tile([65, 512], F32, tag=f"out{hh}")
                                out_AB.append(o)
                            prev_exs = None
                            for kt in range(NTT):
                                exs = emit_scores(hp, cur_chunks, kt)
                                for _ in range(3):
                                    next(next_gen, None)
                                if prev_exs is not None:
                                    for hh in range(2):
                                        h = hp * 2 + hh
                                        nc.tensor.matmul(
                                            out_AB[hh], v_sb[kt - 1][:, h % 8, :],
                                            prev_exs[hh],
                                            start=(kt - 1 == 0), stop=False)
                                prev_exs = exs
                            for hh in range(2):
                                h = hp * 2 + hh
                                nc.tensor.matmul(
                                    out_AB[hh], v_sb[NTT - 1][:, h % 8, :],
                                    prev_exs[hh], start=False, stop=True)
                            for _ in next_gen:
                                pass
                            for hh in range(2):
                                p0 = 64 * hh
                                out_ps = out_AB[hh]
                                rr0 = rpool.tile([1, 512], F32, tag="rr0")
                                nc.vector.tensor_copy(rr0, out_ps[64:65, :])
                                rr = rpool.tile([1, 512], F32, tag="rr")
                                nc.vector.reciprocal_approx_fast(out=rr, in_=rr0)
                                rr_r = rpool.tile([1, 512], F32R, tag="rr_r")
                                nc.scalar.activation(rr_r, rr, ACTF.Copy)
                                bc = bps.tile([64, 512], F32, tag="bc")
                                nc.tensor.matmul(bc, ones_row[:, 0:64], rr_r,
                                                 start=True, stop=True)
                                bc_sb = fpool.tile([64, 512], F32, tag="bcs")
                                nc.vector.tensor_copy(bc_sb, bc)
                                t1 = fpool.tile([64, 512], F32, tag="yt")
                                nc.vector.tensor_mul(t1, out_ps[0:64, :], bc_sb)
                                nc.vector.tensor_scalar_add(
                                    yT[hp][p0:p0 + 64, :], t1,
                                    col(CP_BV + hp)[p0:p0 + 64, :])
                            cur_chunks = next_chunks

            # -------- Phase 5: attention out proj + residual --------
            with ExitStack() as st:
                xrp = st.enter_context(tc.tile_pool(name="xrp", bufs=1))
                wpool = st.enter_context(tc.tile_pool(name="wo", bufs=3))
                pps = st.enter_context(tc.tile_pool(name="ops2", bufs=1, space="PSUM"))
                tpool = st.enter_context(tc.tile_pool(name="t5", bufs=2))
                yT_r = []
                for ft in range(NCT):
                    r = xrp.tile([128, TQ], F32R, tag=f"yTr{ft}")
                    nc.scalar.activation(r, yT[ft], ACTF.Copy)
                    yT_r.append(r)
                xq_res = []
                for ct in range(NCT):
                    xr = xrp.tile([128, TQ], F32, tag=f"xqres{ct}")
                    nc.sync.dma_start(out=xr,
                                      in_=xqT.ap()[ct * 128:(ct + 1) * 128, :])
                    xq_res.append(xr)
                for grp in range(2):
                    pss = []
                    for f4 in range(4):
                        ps = pps.tile([128, TQ], F32, tag=f"ps{f4}")
                        pss.append(ps)
                    for ct in range(NCT):
                        wt = wpool.tile([128, 512], F32R, tag="w")
                        nc.sync.dma_start(out=wt, in_=wo_p.ap()[ct, grp].bitcast(F32R))
                        for f4 in range(4):
                            nc.tensor.matmul(pss[f4], wt[:, f4 * 128:f4 * 128 + 128],
                                             yT_r[ct], start=(ct == 0),
                                             stop=(ct == NCT - 1))
                    for f4 in range(4):
                        ft = grp * 4 + f4
                        t = tpool.tile([128, TQ], F32, tag="t")
                        nc.vector.tensor_add(t, pss[f4], xq_res[ft])
                        x2 = tpool.tile([128, TQ], F32, tag="x2")
                        nc.scalar.activation(x2, t, ACTF.Identity,
                                             bias=col(CP_BO + ft))
                        nc.sync.dma_start(
                            out=x2_d.ap()[ft * 128:(ft + 1) * 128, :], in_=x2)

        sp1.close()  # free raw-x/qT/yT region before the MLP phases

        # ---------------- Phase 6-8: LN2 + MLP ----------------
        gpool = top.enter_context(tc.tile_pool(name="gpool", bufs=1))
        x2p = top.enter_context(tc.tile_pool(name="x2p", bufs=1))
        x2T = []
        for ct in range(NCT):
            x2t_t = x2p.tile([128, TQ], F32R, tag=f"x2L{ct}")
            nc.sync.dma_start(
                out=x2t_t, in_=x2_d.ap()[ct * 128:(ct + 1) * 128, :].bitcast(F32R))
            x2T.append(x2t_t)
        with ExitStack() as stg:
            hpool = stg.enter_context(tc.tile_pool(name="hpool", bufs=1))
            with ExitStack() as st:
                pools2 = make_ln_pools(st, "ln2")
                abc2 = ln_stats(x2T, TQ, "H", pools2, pools2["bc"])
                hT = ln_apply(x2T, TQ, "H", CP_G2, CP_BL2, hpool, "hT",
                              pools2, abc2)
            gT = []
            with ExitStack() as st:
                wpool = st.enter_context(tc.tile_pool(name="w1", bufs=4))
                pps = st.enter_context(tc.tile_pool(name="m1ps", bufs=1, space="PSUM"))
                for grp in range(8):
                    pss = []
                    for f4 in range(4):
                        ps = pps.tile([128, TQ], F32, tag=f"ps{f4}")
                        pss.append(ps)
                    for ct in range(NCT):
                        wt = wpool.tile([128, 512], F32R, tag="w")
                        nc.sync.dma_start(out=wt, in_=w1_p.ap()[ct, grp].bitcast(F32R))
                        for f4 in range(4):
                            nc.tensor.matmul(pss[f4], wt[:, f4 * 128:f4 * 128 + 128],
                                             hT[ct][0], start=(ct == 0),
                                             stop=(ct == NCT - 1))
                    for f4 in range(4):
                        hf = grp * 4 + f4
                        g = gpool.tile([128, TQ], F32R, tag=f"gT{hf}")
                        nc.scalar.activation(g, pss[f4], ACTF.Gelu,
                                             bias=col(CP_B1 + hf))
                        gT.append(g)

        with ExitStack() as st:
            wpool = st.enter_context(tc.tile_pool(name="w2", bufs=4))
            pps = st.enter_context(tc.tile_pool(name="m2ps", bufs=1, space="PSUM"))
            tpool = st.enter_context(tc.tile_pool(name="t8", bufs=3))
            for grp in range(2):
                pss = []
                for f4 in range(4):
                    ps = pps.tile([128, TQ], F32, tag=f"ps{f4}")
                    pss.append(ps)
                for hf in range(NHT):
                    wt = wpool.tile([128, 512], F32R, tag="w")
                    nc.sync.dma_start(out=wt, in_=w2_p.ap()[hf, grp].bitcast(F32R))
                    for f4 in range(4):
                        nc.tensor.matmul(pss[f4], wt[:, f4 * 128:f4 * 128 + 128],
                                         gT[hf], start=(hf == 0),
                                         stop=(hf == NHT - 1))
                for f4 in range(4):
                    ft = grp * 4 + f4
                    t = tpool.tile([128, TQ], F32, tag="t")
                    nc.scalar.activation(t, pss[f4], ACTF.Identity,
                                         bias=col(CP_B2 + ft))
                    o = tpool.tile([128, TQ], F32, tag="o")
                    nc.vector.tensor_add(o, t, x2T[ft].bitcast(F32))
                    nc.sync.dma_start(out=out_d.ap()[ft * 128:(ft + 1) * 128, :],
                                      in_=o)

    nc.compile()
    return nc


def _prep_inputs(inputs):
    f64 = np.float64
    x = np.asarray(inputs["x"], np.float32)
    g1 = np.asarray(inputs["ln1_g"], f64)
    b1v = np.asarray(inputs["ln1_b"], f64)
    Wq = np.asarray(inputs["Wq"], f64) * g1[:, None]
    Wk = np.asarray(inputs["Wk"], f64) * g1[:, None]
    Wv = np.asarray(inputs["Wv"], f64) * g1[:, None]
    bq8_eff = 0.125 * (b1v @ np.asarray(inputs["Wq"], f64)
                       + np.asarray(inputs["bq"], f64))
    bk_eff = b1v @ np.asarray(inputs["Wk"], f64) + np.asarray(inputs["bk"], f64)
    bv_eff = b1v @ np.asarray(inputs["Wv"], f64) + np.asarray(inputs["bv"], f64)
    colWq8 = 0.125 * Wq.sum(0)
    colWk = Wk.sum(0)
    rowWv = Wv.sum(0)

    common = dict(
        wq_p=_grid2((0.125 * Wq).astype(np.float32)),
        wk_g=_grid(Wk.astype(np.float32)),
        wv=np.ascontiguousarray(Wv.astype(np.float32)),
        wo_p=_grid4(np.asarray(inputs["Wo"], np.float32)),
        w1_p=_grid4(np.asarray(inputs["W1"], np.float32)),
        w2_p=_grid4(np.asarray(inputs["W2"], np.float32)),
        rowwv=rowWv.astype(np.float32),
    )
    cpk = np.zeros((128, CP_N), np.float32)
    cpk[:, CP_BQ8:CP_BQ8 + 8] = _pack_cols(bq8_eff)
    cpk[:, CP_BK:CP_BK + 8] = _pack_cols(bk_eff)
    cpk[:, CP_BO:CP_BO + 8] = _pack_cols(np.asarray(inputs["bo"], np.float32))
    cpk[:, CP_B2:CP_B2 + 8] = _pack_cols(np.asarray(inputs["b2"], np.float32))
    cpk[:, CP_G2:CP_G2 + 8] = _pack_cols(np.asarray(inputs["ln2_g"], np.float32))
    cpk[:, CP_BL2:CP_BL2 + 8] = _pack_cols(np.asarray(inputs["ln2_b"], np.float32))
    cpk[:, CP_BV:CP_BV + 8] = _pack_cols(bv_eff)
    cpk[:, CP_B1:CP_B1 + 32] = _pack_cols(np.asarray(inputs["b1"], np.float32))
    cpk[:, CP_EPS] = LN_EPS
    cpk[:, CP_NEG1] = -1.0
    cpk[:, CP_QSCL] = 0.125
    cpk[:, CP_CWK:CP_CWK + 8] = _pack_cols(colWk)
    cpk[:, CP_CWQ8:CP_CWQ8 + 8] = _pack_cols(colWq8)
    common["colpack"] = cpk

    in_maps = []
    for core in range(N_CORES):
        b, s = divmod(core, N_CORES // B)
        m = dict(common)
        m["xbT"] = np.ascontiguousarray(x[b].T)
        m["xqT"] = np.ascontiguousarray(x[b, s * TQ:(s + 1) * TQ, :].T)
        in_maps.append(m)
    return in_maps


def kernel(**inputs):
    from concourse.bass_utils import run_bass_kernel_spmd
    if "nc" not in _CACHE:
        _CACHE["nc"] = _build_program()
    nc = _CACHE["nc"]
    in_maps = _prep_inputs(inputs)
    res = run_bass_kernel_spmd(nc, in_maps, list(range(N_CORES)))
    _CACHE["last_res"] = res
    out = np.empty((B, T, C), np.float32)
    for core in range(N_CORES):
        b, s = divmod(core, N_CORES // B)
        out[b, s * TQ:(s + 1) * TQ, :] = res.results[core]["outT"].T
    return out

